# revision 13
# baseline (speedup 1.0000x reference)
"""2-layer GCN (message passing) on 8 TRN2 NeuronCores.

Strategy: fold the symmetric GCN normalization into per-row scalings by
dinv = rsqrt(deg), so propagation becomes Q = (A+I)^T @ P' with P' the
dinv-scaled linear outputs.  The (A+I) operator is materialized on host as
dense per-core count shards (dst-sharded, exact in fp8) and kept resident
in SBUF; the propagate is a dense bf16 x fp8 matmul accumulated in fp32
PSUM.  Between layers the node-feature shards are exchanged with 8-core
AllGathers, split into two halves and software-pipelined against the
column-slice structure of the propagate matmul so communication hides
under PE work.

Per core c (owns dst nodes [1250c, 1250(c+1))):
  W-matmul   : P = x_c @ W + b (fp32, node-major PSUM), scale rows by dinv
  AllGather  : bf16 shard halves [512|768 x 128] -> full P' [10240, 128]
  A-matmul   : Q^T[feat, dst] = sum_k P'[k-chunk]^T-as-weights @ A[k, dst],
               column-sliced 512|512|226; epilogue + next phase per slice
  epilogue   : h = relu(Q) * dinv (feature-major, fp32)
Final: L2 row-normalize via ones-matmul column sums, then @ Wc + bc.
"""

import sys

if "/opt/trn_rl_repo" not in sys.path:
    sys.path.insert(0, "/opt/trn_rl_repo")

import numpy as np
import ml_dtypes

N, E, D, H, C = 10000, 640000, 128, 128, 40
NC_ = 8                 # cores
NSH = N // NC_          # 1250 nodes per core
NCH = 10                # 128-row chunks per core shard (padded)
NPAD = NCH * 128        # 1280 padded shard rows
NFULL = NPAD * NC_      # 10240 padded global rows
KCH = NFULL // 128      # 80 source chunks
SLICES = [(0, 512), (512, 512), (1024, NSH - 1024)]
MJ_LAST = NSH - 9 * 128  # 98 real rows in the last chunk
# AllGather halves, 512-row aligned so slice-0 epilogue feeds half-a:
# half a = local chunks 0..3, half b = chunks 4..9
HALF_CHUNKS = [list(range(0, 4)), list(range(4, 10))]
# slice index of each local node chunk
CHUNK_SLICE = [0, 0, 0, 0, 1, 1, 1, 1, 2, 2]

_cache = {}


def _build():
    import concourse.bass as bass  # noqa: F401
    import concourse.bacc as bacc
    import concourse.mybir as mybir
    import concourse.tile as tile

    dt = mybir.dt
    F32, BF16, FP8 = dt.float32, dt.bfloat16, dt.float8e4
    AF = mybir.ActivationFunctionType
    RG = [list(range(NC_))]

    nc = bacc.Bacc("TRN2", target_bir_lowering=False, debug=False, num_devices=NC_)

    xT_d = nc.dram_tensor("xT", [128, NSH], F32, kind="ExternalInput").ap()
    A_d = nc.dram_tensor("A", [128, KCH * NSH], FP8, kind="ExternalInput").ap()
    degpp_d = nc.dram_tensor("degpp", [128, NCH], F32, kind="ExternalInput").ap()
    degrow_d = nc.dram_tensor("degrow", [1, NSH], F32, kind="ExternalInput").ap()
    W1_d = nc.dram_tensor("W1", [H, H], F32, kind="ExternalInput").ap()
    W2_d = nc.dram_tensor("W2", [H, H], F32, kind="ExternalInput").ap()
    Wc_d = nc.dram_tensor("Wc", [H, C], F32, kind="ExternalInput").ap()
    b1_d = nc.dram_tensor("b1", [1, H], F32, kind="ExternalInput").ap()
    b2_d = nc.dram_tensor("b2", [1, H], F32, kind="ExternalInput").ap()
    bc_d = nc.dram_tensor("bc", [1, C], F32, kind="ExternalInput").ap()
    out_d = nc.dram_tensor("out", [NPAD, C], F32, kind="ExternalOutput").ap()

    with tile.TileContext(nc) as tc:
        with (
            tc.tile_pool(name="cst", bufs=1) as cst,
            tc.tile_pool(name="wk", bufs=1) as wk,
            tc.tile_pool(name="hpp", bufs=1) as hpp,
            tc.tile_pool(name="pfm", bufs=1, space="PSUM") as pfm,
            tc.tile_pool(name="pnm", bufs=2, space="PSUM") as pnm,
            tc.tile_pool(name="pns", bufs=1, space="PSUM") as pns_p,
            tc.tile_pool(name="dram", bufs=2, space="DRAM") as dram,
        ):
            # ---- inputs to SBUF; small loads on the Sync DGE ring, the big
            # resident adjacency on the Scalar DGE ring so it doesn't block
            # them (HWDGE is FIFO per ring) ----
            xT = cst.tile([128, NSH], F32, tag="xT")
            nc.sync.dma_start(out=xT[:], in_=xT_d)
            W1s = cst.tile([H, H], F32, tag="W1")
            nc.sync.dma_start(out=W1s[:], in_=W1_d)
            W2s = cst.tile([H, H], F32, tag="W2")
            nc.sync.dma_start(out=W2s[:], in_=W2_d)
            Wcs = cst.tile([H, C], F32, tag="Wc")
            nc.sync.dma_start(out=Wcs[:], in_=Wc_d)
            b1s = cst.tile([1, H], F32, tag="b1")
            nc.sync.dma_start(out=b1s[:], in_=b1_d)
            b2s = cst.tile([1, H], F32, tag="b2")
            nc.sync.dma_start(out=b2s[:], in_=b2_d)
            bcs = cst.tile([1, C], F32, tag="bc")
            nc.sync.dma_start(out=bcs[:], in_=bc_d)
            degpp = wk.tile([128, NCH], F32, tag="degpp")
            nc.sync.dma_start(out=degpp[:], in_=degpp_d)
            degrow = wk.tile([1, NSH], F32, tag="degrow")
            nc.sync.dma_start(out=degrow[:], in_=degrow_d)

            # SWDGE (gpsimd) path: its completion rides the DMA-SW semaphore
            # lanes, so HWDGE waiters aren't queued behind this 12.8MB load
            asb = cst.tile([128, KCH * NSH], FP8, tag="A")
            nc.gpsimd.dma_start(out=asb[:], in_=A_d)

            ones_r = cst.tile([1, 128], F32, tag="ones_r")
            nc.vector.memset(ones_r[:], 1.0)
            ones_c = cst.tile([128, 1], F32, tag="ones_c")
            nc.vector.memset(ones_c[:], 1.0)

            # ---- dinv = 1/sqrt(deg) ----
            dinvpp = cst.tile([128, NCH], F32, tag="dinvpp")
            tmp_pp = wk.tile([128, NCH], F32, tag="tmp_pp")
            nc.scalar.sqrt(tmp_pp[:], degpp[:])
            nc.vector.reciprocal(dinvpp[:], tmp_pp[:])
            dinvrow = cst.tile([1, NSH], F32, tag="dinvrow")
            tmp_row = wk.tile([1, NSH], F32, tag="tmp_row")
            nc.scalar.sqrt(tmp_row[:], degrow[:])
            nc.vector.reciprocal(dinvrow[:], tmp_row[:])

            # dinv broadcast across partitions: rank-1 matmul ones x dinvrow
            dinvbc = cst.tile([128, NSH], F32, tag="dinvbc")
            psb0 = pfm.tile([128, NSH], F32, tag="fm")
            for o, n in SLICES:
                nc.tensor.matmul(
                    out=psb0[:, o : o + n], lhsT=ones_r[:, :],
                    rhs=dinvrow[:, o : o + n], start=True, stop=True,
                )
            nc.scalar.copy(dinvbc[:], psb0[:])

            def w_chunk(inT, Ws, bs, sh, j):
                """One 128-node chunk of P' = dinv*(h@W + b), bf16 into sh."""
                mj = 128 if j < 9 else MJ_LAST
                pj = pnm.tile([128, H], F32, tag="nm")
                nc.tensor.matmul(
                    out=pj[:mj, :], lhsT=inT[:, j * 128 : j * 128 + mj],
                    rhs=Ws[:], start=True, stop=False,
                )
                nc.tensor.matmul(
                    out=pj[:mj, :], lhsT=ones_r[:, :mj], rhs=bs[:],
                    start=False, stop=True,
                )
                nc.scalar.activation(
                    sh[:mj, j * 128 : (j + 1) * 128], pj[:mj, :],
                    AF.Copy, scale=dinvpp[:mj, j : j + 1],
                )

            def ag_half(sh, hf, lnum):
                """Bounce + AllGather one half of the shard, land in SBUF."""
                chunks = HALF_CHUNKS[hf]
                rows = len(chunks) * 128
                o = chunks[0] * 128
                bounce = dram.tile([rows, 128], BF16, tag=f"bounce{lnum}{hf}")
                agf = dram.tile([rows * NC_, 128], BF16, tag=f"agf{lnum}{hf}",
                                addr_space="Shared")
                nc.sync.dma_start(
                    out=bounce[:].rearrange("(j p) f -> p j f", p=128),
                    in_=sh[:, o : o + rows].rearrange("p (j f) -> p j f", f=128),
                )
                nc.gpsimd.collective_compute(
                    "AllGather", mybir.AluOpType.bypass, replica_groups=RG,
                    ins=[bounce.opt()], outs=[agf.opt()],
                )
                hp = hpp.tile([128, rows * NC_], BF16, tag=f"hp{hf}")
                # per-rank pieces so downstream matmuls start on the first
                # ranks while later pieces are still landing
                for c in range(NC_):
                    nc.sync.dma_start(
                        out=hp[:, c * rows : (c + 1) * rows].rearrange(
                            "p (q f) -> p q f", f=128),
                        in_=agf[c * rows : (c + 1) * rows].rearrange(
                            "(q p) f -> p q f", p=128),
                    )
                return hp

            # A-matmul source-chunk (k = global chunk, q = index in half)
            # orders for the two halves.
            HALF_KQ = [
                [(NCH * c + j, len(HALF_CHUNKS[hf]) * c + ji)
                 for c in range(NC_)
                 for ji, j in enumerate(HALF_CHUNKS[hf])]
                for hf in range(2)
            ]

            def a_phase_a(hps, ps):
                """Half-a chunks, all 3 slices, chunk-major: ~21us of PE work
                available as soon as AllGather half-a lands."""
                for i, (k, q) in enumerate(HALF_KQ[0]):
                    for o, n in SLICES:
                        nc.tensor.matmul(
                            out=ps[:, o : o + n],
                            lhsT=hps[0][:, q * 128 : (q + 1) * 128],
                            rhs=asb[:, k * NSH + o : k * NSH + o + n],
                            start=(i == 0), stop=False,
                        )

            def a_phase_b_slice(hps, ps, s):
                """Finish slice s with the half-b chunks."""
                o, n = SLICES[s]
                last = len(HALF_KQ[1]) - 1
                for i, (k, q) in enumerate(HALF_KQ[1]):
                    nc.tensor.matmul(
                        out=ps[:, o : o + n],
                        lhsT=hps[1][:, q * 128 : (q + 1) * 128],
                        rhs=asb[:, k * NSH + o : k * NSH + o + n],
                        start=False, stop=(i == last),
                    )

            def epilogue_slice(ps, hT, s):
                """h[:, slice] = relu(Q) * dinv for one column slice."""
                o, n = SLICES[s]
                rel = wk.tile([128, NSH], F32, tag="rel")
                nc.scalar.activation(rel[:, o : o + n], ps[:, o : o + n], AF.Relu)
                nc.vector.tensor_mul(
                    hT[:, o : o + n], rel[:, o : o + n], dinvbc[:, o : o + n]
                )

            # ---- layer 1 W phase + first AllGathers ----
            sh1 = wk.tile([128, NPAD], BF16, tag="sh1")
            nc.vector.memset(sh1[:, 9 * 128 : NPAD], 0.0)
            for j in HALF_CHUNKS[0]:
                w_chunk(xT, W1s, b1s, sh1, j)
            hp_a = ag_half(sh1, 0, 1)
            for j in HALF_CHUNKS[1]:
                w_chunk(xT, W1s, b1s, sh1, j)
            hp_b = ag_half(sh1, 1, 1)
            hps1 = [hp_a, hp_b]

            # ---- layer 1 propagate, pipelined with layer 2 W phase ----
            ps1 = pfm.tile([128, NSH], F32, tag="fm")
            h1T = wk.tile([128, NSH], F32, tag="h1T")
            sh2 = wk.tile([128, NPAD], BF16, tag="sh2")
            nc.vector.memset(sh2[:, 9 * 128 : NPAD], 0.0)
            hps2 = [None, None]
            a_phase_a(hps1, ps1)
            for s in range(3):
                a_phase_b_slice(hps1, ps1, s)
                epilogue_slice(ps1, h1T, s)
                for j in range(NCH):
                    if CHUNK_SLICE[j] == s:
                        w_chunk(h1T, W2s, b2s, sh2, j)
                if s == 0:
                    hps2[0] = ag_half(sh2, 0, 2)
            hps2[1] = ag_half(sh2, 1, 2)

            # ---- layer 2 propagate + full per-slice finalization ----
            # Per column slice: propagate, epilogue, squared column sums,
            # rsqrt normalization, classifier — so nearly the whole tail
            # overlaps the remaining propagate slices.
            ps2 = pfm.tile([128, NSH], F32, tag="fm")
            h2T = wk.tile([128, NSH], F32, tag="h2T")
            sq = wk.tile([128, NSH], F32, tag="sq")
            pns = pns_p.tile([1, NSH], F32, tag="ns")
            sr = wk.tile([1, NSH], F32, tag="sr")
            rr = wk.tile([1, NSH], F32, tag="rr")
            psb = pfm.tile([128, NSH], F32, tag="fm")
            hn = wk.tile([128, NSH], F32, tag="hn")
            oc = wk.tile([128, NCH * C], F32, tag="oc")
            nc.vector.memset(oc[:, 9 * C : NCH * C], 0.0)
            a_phase_a(hps2, ps2)
            for s in range(3):
                o, n = SLICES[s]
                a_phase_b_slice(hps2, ps2, s)
                epilogue_slice(ps2, h2T, s)
                nc.scalar.square(sq[:, o : o + n], h2T[:, o : o + n])
                nc.tensor.matmul(
                    out=pns[:, o : o + n], lhsT=ones_c[:, :],
                    rhs=sq[:, o : o + n], start=True, stop=True,
                )
                nc.scalar.sqrt(sr[:, o : o + n], pns[:, o : o + n])
                nc.vector.tensor_scalar_max(
                    sr[:, o : o + n], sr[:, o : o + n], 1e-12
                )
                nc.vector.reciprocal(rr[:, o : o + n], sr[:, o : o + n])
                nc.tensor.matmul(
                    out=psb[:, o : o + n], lhsT=ones_r[:, :],
                    rhs=rr[:, o : o + n], start=True, stop=True,
                )
                nc.vector.tensor_mul(
                    hn[:, o : o + n], h2T[:, o : o + n], psb[:, o : o + n]
                )
                for j in range(NCH):
                    if CHUNK_SLICE[j] != s:
                        continue
                    mj = 128 if j < 9 else MJ_LAST
                    pc = pnm.tile([128, H], F32, tag="nm")
                    nc.tensor.matmul(
                        out=pc[:mj, :C], lhsT=hn[:, j * 128 : j * 128 + mj],
                        rhs=Wcs[:], start=True, stop=False,
                    )
                    nc.tensor.matmul(
                        out=pc[:mj, :C], lhsT=ones_r[:, :mj], rhs=bcs[:],
                        start=False, stop=True,
                    )
                    nc.scalar.copy(oc[:mj, j * C : (j + 1) * C], pc[:mj, :C])
            nc.sync.dma_start(
                out=out_d.rearrange("(j p) c -> p j c", p=128),
                in_=oc[:].rearrange("p (j c) -> p j c", c=C),
            )

    nc.compile()
    return nc


def _prep(inputs):
    x = np.asarray(inputs["x"], np.float32)
    ei = np.asarray(inputs["edge_index"])
    src = ei[0].astype(np.int64)
    dst = ei[1].astype(np.int64)
    loops = np.arange(N, dtype=np.int64)
    s_all = np.concatenate([src, loops])
    d_all = np.concatenate([dst, loops])
    deg = np.bincount(d_all, minlength=N).astype(np.float32)
    gsrc = (s_all // NSH) * NPAD + (s_all % NSH)  # padded global row of source
    dcore = d_all // NSH
    dloc = d_all % NSH

    W1 = np.ascontiguousarray(np.asarray(inputs["W1"], np.float32))
    W2 = np.ascontiguousarray(np.asarray(inputs["W2"], np.float32))
    Wc = np.ascontiguousarray(np.asarray(inputs["Wc"], np.float32))
    b1 = np.asarray(inputs["b1"], np.float32).reshape(1, H)
    b2 = np.asarray(inputs["b2"], np.float32).reshape(1, H)
    bc = np.asarray(inputs["bc"], np.float32).reshape(1, C)

    in_maps = []
    for c in range(NC_):
        m = dcore == c
        flat = gsrc[m] * NSH + dloc[m]
        Ac = np.bincount(flat, minlength=NFULL * NSH).astype(np.float32)
        Ac = Ac.reshape(KCH, 128, NSH).transpose(1, 0, 2)
        Ac = np.ascontiguousarray(Ac).reshape(128, KCH * NSH)
        degc = deg[c * NSH : (c + 1) * NSH]
        degpp = np.concatenate([degc, np.ones(NPAD - NSH, np.float32)])
        degpp = np.ascontiguousarray(degpp.reshape(NCH, 128).T)
        in_maps.append({
            "xT": np.ascontiguousarray(x[c * NSH : (c + 1) * NSH].T),
            "A": Ac.astype(ml_dtypes.float8_e4m3),
            "degpp": degpp,
            "degrow": np.ascontiguousarray(degc.reshape(1, NSH)),
            "W1": W1, "W2": W2, "Wc": Wc,
            "b1": b1, "b2": b2, "bc": bc,
        })
    return in_maps


def run(inputs, **spmd_kwargs):
    from concourse import bass_utils

    if "nc" not in _cache:
        _cache["nc"] = _build()
    in_maps = _prep(inputs)
    res = bass_utils.run_bass_kernel_spmd(
        _cache["nc"], in_maps, core_ids=list(range(NC_)), **spmd_kwargs
    )
    out = np.concatenate(
        [np.asarray(res.results[c]["out"])[:NSH] for c in range(NC_)], axis=0
    )
    return out.astype(np.float32), res


def kernel(**inputs):
    out, _ = run(inputs)
    return out


# revision 15
# speedup vs baseline: 1.0050x; 1.0050x over previous
"""2-layer GCN (message passing) on 8 TRN2 NeuronCores.

Strategy: fold the symmetric GCN normalization into per-row scalings by
dinv = rsqrt(deg), so propagation becomes Q = (A+I)^T @ P' with P' the
dinv-scaled linear outputs.  The (A+I) operator is materialized on host as
dense per-core count shards (dst-sharded, exact in fp8) and kept resident
in SBUF; the propagate is a dense bf16 x fp8 matmul accumulated in fp32
PSUM.  Between layers the node-feature shards are exchanged with 8-core
AllGathers, split into two halves and software-pipelined against the
column-slice structure of the propagate matmul so communication hides
under PE work.

Per core c (owns dst nodes [1250c, 1250(c+1))):
  W-matmul   : P = x_c @ W + b (fp32, node-major PSUM), scale rows by dinv
  AllGather  : bf16 shard halves [512|768 x 128] -> full P' [10240, 128]
  A-matmul   : Q^T[feat, dst] = sum_k P'[k-chunk]^T-as-weights @ A[k, dst],
               column-sliced 512|512|226; epilogue + next phase per slice
  epilogue   : h = relu(Q) * dinv (feature-major, fp32)
Final: L2 row-normalize via ones-matmul column sums, then @ Wc + bc.
"""

import sys

if "/opt/trn_rl_repo" not in sys.path:
    sys.path.insert(0, "/opt/trn_rl_repo")

import numpy as np
import ml_dtypes

N, E, D, H, C = 10000, 640000, 128, 128, 40
NC_ = 8                 # cores
NSH = N // NC_          # 1250 nodes per core
NCH = 10                # 128-row chunks per core shard (padded)
NPAD = NCH * 128        # 1280 padded shard rows
NFULL = NPAD * NC_      # 10240 padded global rows
KCH = NFULL // 128      # 80 source chunks
SLICES = [(0, 512), (512, 512), (1024, NSH - 1024)]
MJ_LAST = NSH - 9 * 128  # 98 real rows in the last chunk
# AllGather halves, 512-row aligned so slice-0 epilogue feeds half-a:
# half a = local chunks 0..3, half b = chunks 4..9
HALF_CHUNKS = [list(range(0, 4)), list(range(4, 10))]
# slice index of each local node chunk
CHUNK_SLICE = [0, 0, 0, 0, 1, 1, 1, 1, 2, 2]

_cache = {}


def _build():
    import concourse.bass as bass  # noqa: F401
    import concourse.bacc as bacc
    import concourse.mybir as mybir
    import concourse.tile as tile

    dt = mybir.dt
    F32, BF16, FP8 = dt.float32, dt.bfloat16, dt.float8e4
    AF = mybir.ActivationFunctionType
    RG = [list(range(NC_))]

    nc = bacc.Bacc("TRN2", target_bir_lowering=False, debug=False, num_devices=NC_)

    xT_d = nc.dram_tensor("xT", [128, NSH], F32, kind="ExternalInput").ap()
    A_d = nc.dram_tensor("A", [128, KCH * NSH], FP8, kind="ExternalInput").ap()
    degpp_d = nc.dram_tensor("degpp", [128, NCH], F32, kind="ExternalInput").ap()
    degrow_d = nc.dram_tensor("degrow", [1, NSH], F32, kind="ExternalInput").ap()
    W1_d = nc.dram_tensor("W1", [H, H], F32, kind="ExternalInput").ap()
    W2_d = nc.dram_tensor("W2", [H, H], F32, kind="ExternalInput").ap()
    Wc_d = nc.dram_tensor("Wc", [H, C], F32, kind="ExternalInput").ap()
    b1_d = nc.dram_tensor("b1", [1, H], F32, kind="ExternalInput").ap()
    b2_d = nc.dram_tensor("b2", [1, H], F32, kind="ExternalInput").ap()
    bc_d = nc.dram_tensor("bc", [1, C], F32, kind="ExternalInput").ap()
    out_d = nc.dram_tensor("out", [NPAD, C], F32, kind="ExternalOutput").ap()

    with tile.TileContext(nc) as tc:
        with (
            tc.tile_pool(name="cst", bufs=1) as cst,
            tc.tile_pool(name="wk", bufs=1) as wk,
            tc.tile_pool(name="hpp", bufs=1) as hpp,
            tc.tile_pool(name="pfm", bufs=1, space="PSUM") as pfm,
            tc.tile_pool(name="pnm", bufs=2, space="PSUM") as pnm,
            tc.tile_pool(name="pns", bufs=1, space="PSUM") as pns_p,
            tc.tile_pool(name="dram", bufs=2, space="DRAM") as dram,
        ):
            # ---- inputs to SBUF; small loads on the Sync DGE ring, the big
            # resident adjacency on the Scalar DGE ring so it doesn't block
            # them (HWDGE is FIFO per ring) ----
            xT = cst.tile([128, NSH], F32, tag="xT")
            nc.sync.dma_start(out=xT[:], in_=xT_d)
            W1s = cst.tile([H, H], F32, tag="W1")
            nc.sync.dma_start(out=W1s[:], in_=W1_d)
            W2s = cst.tile([H, H], F32, tag="W2")
            nc.sync.dma_start(out=W2s[:], in_=W2_d)
            Wcs = cst.tile([H, C], F32, tag="Wc")
            nc.sync.dma_start(out=Wcs[:], in_=Wc_d)
            b1s = cst.tile([1, H], F32, tag="b1")
            nc.sync.dma_start(out=b1s[:], in_=b1_d)
            b2s = cst.tile([1, H], F32, tag="b2")
            nc.sync.dma_start(out=b2s[:], in_=b2_d)
            bcs = cst.tile([1, C], F32, tag="bc")
            nc.sync.dma_start(out=bcs[:], in_=bc_d)
            degpp = wk.tile([128, NCH], F32, tag="degpp")
            nc.sync.dma_start(out=degpp[:], in_=degpp_d)
            degrow = wk.tile([1, NSH], F32, tag="degrow")
            nc.sync.dma_start(out=degrow[:], in_=degrow_d)

            asb = cst.tile([128, KCH * NSH], FP8, tag="A")

            ones_r = cst.tile([1, 128], F32, tag="ones_r")
            nc.vector.memset(ones_r[:], 1.0)
            ones_c = cst.tile([128, 1], F32, tag="ones_c")
            nc.vector.memset(ones_c[:], 1.0)

            # ---- dinv = 1/sqrt(deg) ----
            dinvpp = cst.tile([128, NCH], F32, tag="dinvpp")
            tmp_pp = wk.tile([128, NCH], F32, tag="tmp_pp")
            nc.scalar.sqrt(tmp_pp[:], degpp[:])
            nc.vector.reciprocal(dinvpp[:], tmp_pp[:])
            dinvrow = cst.tile([1, NSH], F32, tag="dinvrow")
            tmp_row = wk.tile([1, NSH], F32, tag="tmp_row")
            nc.scalar.sqrt(tmp_row[:], degrow[:])
            nc.vector.reciprocal(dinvrow[:], tmp_row[:])

            # dinv broadcast across partitions: rank-1 matmul ones x dinvrow
            dinvbc = cst.tile([128, NSH], F32, tag="dinvbc")
            psb0 = pfm.tile([128, NSH], F32, tag="fm")
            for o, n in SLICES:
                nc.tensor.matmul(
                    out=psb0[:, o : o + n], lhsT=ones_r[:, :],
                    rhs=dinvrow[:, o : o + n], start=True, stop=True,
                )
            nc.scalar.copy(dinvbc[:], psb0[:])

            def w_chunk(inT, Ws, bs, sh, j):
                """One 128-node chunk of P' = dinv*(h@W + b), bf16 into sh."""
                mj = 128 if j < 9 else MJ_LAST
                pj = pnm.tile([128, H], F32, tag="nm")
                nc.tensor.matmul(
                    out=pj[:mj, :], lhsT=inT[:, j * 128 : j * 128 + mj],
                    rhs=Ws[:], start=True, stop=False,
                )
                nc.tensor.matmul(
                    out=pj[:mj, :], lhsT=ones_r[:, :mj], rhs=bs[:],
                    start=False, stop=True,
                )
                nc.scalar.activation(
                    sh[:mj, j * 128 : (j + 1) * 128], pj[:mj, :],
                    AF.Copy, scale=dinvpp[:mj, j : j + 1],
                )

            def ag_half(sh, hf, lnum):
                """Bounce + AllGather one half of the shard, land in SBUF."""
                chunks = HALF_CHUNKS[hf]
                rows = len(chunks) * 128
                o = chunks[0] * 128
                bounce = dram.tile([rows, 128], BF16, tag=f"bounce{lnum}{hf}")
                agf = dram.tile([rows * NC_, 128], BF16, tag=f"agf{lnum}{hf}",
                                addr_space="Shared")
                nc.sync.dma_start(
                    out=bounce[:].rearrange("(j p) f -> p j f", p=128),
                    in_=sh[:, o : o + rows].rearrange("p (j f) -> p j f", f=128),
                )
                nc.gpsimd.collective_compute(
                    "AllGather", mybir.AluOpType.bypass, replica_groups=RG,
                    ins=[bounce.opt()], outs=[agf.opt()],
                )
                hp = hpp.tile([128, rows * NC_], BF16, tag=f"hp{hf}")
                # per-rank pieces so downstream matmuls start on the first
                # ranks while later pieces are still landing
                for c in range(NC_):
                    nc.sync.dma_start(
                        out=hp[:, c * rows : (c + 1) * rows].rearrange(
                            "p (q f) -> p q f", f=128),
                        in_=agf[c * rows : (c + 1) * rows].rearrange(
                            "(q p) f -> p q f", p=128),
                    )
                return hp

            # A-matmul source-chunk (k = global chunk, q = index in half)
            # orders for the two halves.
            HALF_KQ = [
                [(NCH * c + j, len(HALF_CHUNKS[hf]) * c + ji)
                 for c in range(NC_)
                 for ji, j in enumerate(HALF_CHUNKS[hf])]
                for hf in range(2)
            ]

            def a_phase_a(hps, ps):
                """Half-a chunks, all 3 slices, chunk-major: ~21us of PE work
                available as soon as AllGather half-a lands."""
                for i, (k, q) in enumerate(HALF_KQ[0]):
                    for o, n in SLICES:
                        nc.tensor.matmul(
                            out=ps[:, o : o + n],
                            lhsT=hps[0][:, q * 128 : (q + 1) * 128],
                            rhs=asb[:, k * NSH + o : k * NSH + o + n],
                            start=(i == 0), stop=False,
                        )

            def a_phase_b_slice(hps, ps, s):
                """Finish slice s with the half-b chunks."""
                o, n = SLICES[s]
                last = len(HALF_KQ[1]) - 1
                for i, (k, q) in enumerate(HALF_KQ[1]):
                    nc.tensor.matmul(
                        out=ps[:, o : o + n],
                        lhsT=hps[1][:, q * 128 : (q + 1) * 128],
                        rhs=asb[:, k * NSH + o : k * NSH + o + n],
                        start=False, stop=(i == last),
                    )

            def epilogue_slice(ps, hT, s):
                """h[:, slice] = relu(Q) * dinv for one column slice."""
                o, n = SLICES[s]
                rel = wk.tile([128, NSH], F32, tag="rel")
                nc.scalar.activation(rel[:, o : o + n], ps[:, o : o + n], AF.Relu)
                nc.vector.tensor_mul(
                    hT[:, o : o + n], rel[:, o : o + n], dinvbc[:, o : o + n]
                )

            # ---- layer 1 W phase + first AllGathers ----
            sh1 = wk.tile([128, NPAD], BF16, tag="sh1")
            nc.vector.memset(sh1[:, 9 * 128 : NPAD], 0.0)
            for j in HALF_CHUNKS[0]:
                w_chunk(xT, W1s, b1s, sh1, j)
            hp_a = ag_half(sh1, 0, 1)
            for j in HALF_CHUNKS[1]:
                w_chunk(xT, W1s, b1s, sh1, j)
            hp_b = ag_half(sh1, 1, 1)
            hps1 = [hp_a, hp_b]

            # Resident adjacency load, traced here on the Scalar DGE ring:
            # late enough that no early waiter's DMA-sem threshold includes
            # this 12.8MB transfer, early enough (right after the W1
            # epilogues in the ACT stream) to land before the propagate.
            nc.scalar.dma_start(out=asb[:], in_=A_d)

            # ---- layer 1 propagate, pipelined with layer 2 W phase ----
            ps1 = pfm.tile([128, NSH], F32, tag="fm")
            h1T = wk.tile([128, NSH], F32, tag="h1T")
            sh2 = wk.tile([128, NPAD], BF16, tag="sh2")
            nc.vector.memset(sh2[:, 9 * 128 : NPAD], 0.0)
            hps2 = [None, None]
            a_phase_a(hps1, ps1)
            for s in range(3):
                a_phase_b_slice(hps1, ps1, s)
                epilogue_slice(ps1, h1T, s)
                for j in range(NCH):
                    if CHUNK_SLICE[j] == s:
                        w_chunk(h1T, W2s, b2s, sh2, j)
                if s == 0:
                    hps2[0] = ag_half(sh2, 0, 2)
            hps2[1] = ag_half(sh2, 1, 2)

            # ---- layer 2 propagate + full per-slice finalization ----
            # Per column slice: propagate, epilogue, squared column sums,
            # rsqrt normalization, classifier — so nearly the whole tail
            # overlaps the remaining propagate slices.
            ps2 = pfm.tile([128, NSH], F32, tag="fm")
            h2T = wk.tile([128, NSH], F32, tag="h2T")
            sq = wk.tile([128, NSH], F32, tag="sq")
            pns = pns_p.tile([1, NSH], F32, tag="ns")
            sr = wk.tile([1, NSH], F32, tag="sr")
            rr = wk.tile([1, NSH], F32, tag="rr")
            psb = pfm.tile([128, NSH], F32, tag="fm")
            hn = wk.tile([128, NSH], F32, tag="hn")
            oc = wk.tile([128, NCH * C], F32, tag="oc")
            nc.vector.memset(oc[:, 9 * C : NCH * C], 0.0)
            a_phase_a(hps2, ps2)
            for s in range(3):
                o, n = SLICES[s]
                a_phase_b_slice(hps2, ps2, s)
                epilogue_slice(ps2, h2T, s)
                nc.scalar.square(sq[:, o : o + n], h2T[:, o : o + n])
                nc.tensor.matmul(
                    out=pns[:, o : o + n], lhsT=ones_c[:, :],
                    rhs=sq[:, o : o + n], start=True, stop=True,
                )
                nc.scalar.sqrt(sr[:, o : o + n], pns[:, o : o + n])
                nc.vector.tensor_scalar_max(
                    sr[:, o : o + n], sr[:, o : o + n], 1e-12
                )
                nc.vector.reciprocal(rr[:, o : o + n], sr[:, o : o + n])
                nc.tensor.matmul(
                    out=psb[:, o : o + n], lhsT=ones_r[:, :],
                    rhs=rr[:, o : o + n], start=True, stop=True,
                )
                nc.vector.tensor_mul(
                    hn[:, o : o + n], h2T[:, o : o + n], psb[:, o : o + n]
                )
                for j in range(NCH):
                    if CHUNK_SLICE[j] != s:
                        continue
                    mj = 128 if j < 9 else MJ_LAST
                    pc = pnm.tile([128, H], F32, tag="nm")
                    nc.tensor.matmul(
                        out=pc[:mj, :C], lhsT=hn[:, j * 128 : j * 128 + mj],
                        rhs=Wcs[:], start=True, stop=False,
                    )
                    nc.tensor.matmul(
                        out=pc[:mj, :C], lhsT=ones_r[:, :mj], rhs=bcs[:],
                        start=False, stop=True,
                    )
                    nc.scalar.copy(oc[:mj, j * C : (j + 1) * C], pc[:mj, :C])
            nc.sync.dma_start(
                out=out_d.rearrange("(j p) c -> p j c", p=128),
                in_=oc[:].rearrange("p (j c) -> p j c", c=C),
            )

    nc.compile()
    return nc


def _prep(inputs):
    x = np.asarray(inputs["x"], np.float32)
    ei = np.asarray(inputs["edge_index"])
    src = ei[0].astype(np.int64)
    dst = ei[1].astype(np.int64)
    loops = np.arange(N, dtype=np.int64)
    s_all = np.concatenate([src, loops])
    d_all = np.concatenate([dst, loops])
    deg = np.bincount(d_all, minlength=N).astype(np.float32)
    gsrc = (s_all // NSH) * NPAD + (s_all % NSH)  # padded global row of source
    dcore = d_all // NSH
    dloc = d_all % NSH

    W1 = np.ascontiguousarray(np.asarray(inputs["W1"], np.float32))
    W2 = np.ascontiguousarray(np.asarray(inputs["W2"], np.float32))
    Wc = np.ascontiguousarray(np.asarray(inputs["Wc"], np.float32))
    b1 = np.asarray(inputs["b1"], np.float32).reshape(1, H)
    b2 = np.asarray(inputs["b2"], np.float32).reshape(1, H)
    bc = np.asarray(inputs["bc"], np.float32).reshape(1, C)

    in_maps = []
    for c in range(NC_):
        m = dcore == c
        flat = gsrc[m] * NSH + dloc[m]
        Ac = np.bincount(flat, minlength=NFULL * NSH).astype(np.float32)
        Ac = Ac.reshape(KCH, 128, NSH).transpose(1, 0, 2)
        Ac = np.ascontiguousarray(Ac).reshape(128, KCH * NSH)
        degc = deg[c * NSH : (c + 1) * NSH]
        degpp = np.concatenate([degc, np.ones(NPAD - NSH, np.float32)])
        degpp = np.ascontiguousarray(degpp.reshape(NCH, 128).T)
        in_maps.append({
            "xT": np.ascontiguousarray(x[c * NSH : (c + 1) * NSH].T),
            "A": Ac.astype(ml_dtypes.float8_e4m3),
            "degpp": degpp,
            "degrow": np.ascontiguousarray(degc.reshape(1, NSH)),
            "W1": W1, "W2": W2, "Wc": Wc,
            "b1": b1, "b2": b2, "bc": bc,
        })
    return in_maps


def run(inputs, **spmd_kwargs):
    from concourse import bass_utils

    if "nc" not in _cache:
        _cache["nc"] = _build()
    in_maps = _prep(inputs)
    res = bass_utils.run_bass_kernel_spmd(
        _cache["nc"], in_maps, core_ids=list(range(NC_)), **spmd_kwargs
    )
    out = np.concatenate(
        [np.asarray(res.results[c]["out"])[:NSH] for c in range(NC_)], axis=0
    )
    return out.astype(np.float32), res


def kernel(**inputs):
    out, _ = run(inputs)
    return out


# revision 20
# speedup vs baseline: 1.0487x; 1.0435x over previous
"""2-layer GCN (message passing) on 8 TRN2 NeuronCores.

Strategy: fold the symmetric GCN normalization into per-row scalings by
dinv = rsqrt(deg), so propagation becomes Q = (A+I)^T @ P' with P' the
dinv-scaled linear outputs.  The (A+I) operator is materialized on host as
dense per-core count shards (dst-sharded, exact in fp8) and kept resident
in SBUF; the propagate is a dense bf16 x fp8 matmul accumulated in fp32
PSUM.  Between layers the node-feature shards are exchanged with 8-core
AllGathers, split into two halves and software-pipelined against the
column-slice structure of the propagate matmul so communication hides
under PE work.

Per core c (owns dst nodes [1250c, 1250(c+1))):
  W-matmul   : P = x_c @ W + b (fp32, node-major PSUM), scale rows by dinv
  AllGather  : bf16 shard halves [512|768 x 128] -> full P' [10240, 128]
  A-matmul   : Q^T[feat, dst] = sum_k P'[k-chunk]^T-as-weights @ A[k, dst],
               column-sliced 512|512|226; epilogue + next phase per slice
  epilogue   : h = relu(Q) * dinv (feature-major, fp32)
Final: L2 row-normalize via ones-matmul column sums, then @ Wc + bc.
"""

import sys

if "/opt/trn_rl_repo" not in sys.path:
    sys.path.insert(0, "/opt/trn_rl_repo")

import numpy as np
import ml_dtypes

N, E, D, H, C = 10000, 640000, 128, 128, 40
NC_ = 8                 # cores
NSH = N // NC_          # 1250 nodes per core
NCH = 10                # 128-row chunks per core shard (padded)
NPAD = NCH * 128        # 1280 padded shard rows
NFULL = NPAD * NC_      # 10240 padded global rows
KCH = NFULL // 128      # 80 source chunks
SLICES = [(0, 512), (512, 512), (1024, NSH - 1024)]
MJ_LAST = NSH - 9 * 128  # 98 real rows in the last chunk
# AllGather halves, 512-row aligned so slice-0 epilogue feeds half-a:
# half a = local chunks 0..3, half b = chunks 4..9
HALF_CHUNKS = [list(range(0, 4)), list(range(4, 10))]
# slice index of each local node chunk
CHUNK_SLICE = [0, 0, 0, 0, 1, 1, 1, 1, 2, 2]
# packed small-input layout (columns of one [128, PK_COLS] f32 tensor)
PK_W1 = 0
PK_W2 = PK_W1 + H          # 128
PK_WC = PK_W2 + H          # 256
PK_DEGPP = PK_WC + C       # 296
PK_B1 = PK_DEGPP + NCH     # 306  (row 0 only)
PK_B2 = PK_B1 + H          # 434  (row 0 only)
PK_BC = PK_B2 + H          # 562  (row 0 only)
PK_DEGROW = PK_BC + C      # 602  (row 0 only)
PK_COLS = PK_DEGROW + NSH  # 1852

_cache = {}


def _build():
    import concourse.bass as bass  # noqa: F401
    import concourse.bacc as bacc
    import concourse.mybir as mybir
    import concourse.tile as tile

    dt = mybir.dt
    F32, BF16, FP8 = dt.float32, dt.bfloat16, dt.float8e4
    AF = mybir.ActivationFunctionType
    RG = [list(range(NC_))]

    nc = bacc.Bacc("TRN2", target_bir_lowering=False, debug=False, num_devices=NC_)

    xT_d = nc.dram_tensor("xT", [128, NSH], F32, kind="ExternalInput").ap()
    A_d = nc.dram_tensor("A", [128, KCH * NSH], FP8, kind="ExternalInput").ap()
    pk_d = nc.dram_tensor("packed", [128, PK_COLS], F32, kind="ExternalInput").ap()
    out_d = nc.dram_tensor("out", [NPAD, C], F32, kind="ExternalOutput").ap()

    with tile.TileContext(nc) as tc:
        with (
            tc.tile_pool(name="cst", bufs=1) as cst,
            tc.tile_pool(name="wk", bufs=1) as wk,
            tc.tile_pool(name="hpp", bufs=1) as hpp,
            tc.tile_pool(name="pfm", bufs=1, space="PSUM") as pfm,
            tc.tile_pool(name="pnm", bufs=2, space="PSUM") as pnm,
            tc.tile_pool(name="pns", bufs=1, space="PSUM") as pns_p,
            tc.tile_pool(name="dram", bufs=2, space="DRAM") as dram,
        ):
            # ---- inputs to SBUF: exactly 3 early DMAs (xT, packed, A) so no
            # two share one of the 8 round-robin DMA completion semaphores —
            # a collision makes early compute wait on the 12.8MB A load ----
            xT = cst.tile([128, NSH], F32, tag="xT")
            nc.sync.dma_start(out=xT[:], in_=xT_d)
            pk = cst.tile([128, PK_COLS], F32, tag="pk")
            nc.sync.dma_start(out=pk[:], in_=pk_d)
            W1s = pk[:, PK_W1 : PK_W1 + H]
            W2s = pk[:, PK_W2 : PK_W2 + H]
            Wcs = pk[:, PK_WC : PK_WC + C]
            degpp = pk[:, PK_DEGPP : PK_DEGPP + NCH]
            b1s = pk[0:1, PK_B1 : PK_B1 + H]
            b2s = pk[0:1, PK_B2 : PK_B2 + H]
            bcs = pk[0:1, PK_BC : PK_BC + C]
            degrow = pk[0:1, PK_DEGROW : PK_DEGROW + NSH]

            asb = cst.tile([128, KCH * NSH], FP8, tag="A")

            ones_r = cst.tile([1, 128], F32, tag="ones_r")
            nc.vector.memset(ones_r[:], 1.0)
            ones_c = cst.tile([128, 1], F32, tag="ones_c")
            nc.vector.memset(ones_c[:], 1.0)

            # ---- dinv = 1/sqrt(deg) ----
            dinvpp = cst.tile([128, NCH], F32, tag="dinvpp")
            tmp_pp = wk.tile([128, NCH], F32, tag="tmp_pp")
            nc.scalar.sqrt(tmp_pp[:], degpp)
            nc.vector.reciprocal(dinvpp[:], tmp_pp[:])
            dinvrow = cst.tile([1, NSH], F32, tag="dinvrow")
            tmp_row = wk.tile([1, NSH], F32, tag="tmp_row")
            nc.scalar.sqrt(tmp_row[:], degrow)
            nc.vector.reciprocal(dinvrow[:], tmp_row[:])

            # dinv broadcast across partitions: rank-1 matmul ones x dinvrow
            dinvbc = cst.tile([128, NSH], F32, tag="dinvbc")
            psb0 = pfm.tile([128, NSH], F32, tag="fm")
            for o, n in SLICES:
                nc.tensor.matmul(
                    out=psb0[:, o : o + n], lhsT=ones_r[:, :],
                    rhs=dinvrow[:, o : o + n], start=True, stop=True,
                )
            nc.scalar.copy(dinvbc[:], psb0[:])

            def w_chunk(inT, Ws, bs, sh, j):
                """One 128-node chunk of P' = dinv*(h@W + b), bf16 into sh."""
                mj = 128 if j < 9 else MJ_LAST
                pj = pnm.tile([128, H], F32, tag="nm")
                nc.tensor.matmul(
                    out=pj[:mj, :], lhsT=inT[:, j * 128 : j * 128 + mj],
                    rhs=Ws, start=True, stop=False,
                )
                nc.tensor.matmul(
                    out=pj[:mj, :], lhsT=ones_r[:, :mj], rhs=bs,
                    start=False, stop=True,
                )
                nc.scalar.activation(
                    sh[:mj, j * 128 : (j + 1) * 128], pj[:mj, :],
                    AF.Copy, scale=dinvpp[:mj, j : j + 1],
                )

            def ag_half(sh, hf, lnum):
                """Bounce + AllGather one half of the shard, land in SBUF."""
                chunks = HALF_CHUNKS[hf]
                rows = len(chunks) * 128
                o = chunks[0] * 128
                bounce = dram.tile([rows, 128], BF16, tag=f"bounce{lnum}{hf}")
                agf = dram.tile([rows * NC_, 128], BF16, tag=f"agf{lnum}{hf}",
                                addr_space="Shared")
                nc.sync.dma_start(
                    out=bounce[:].rearrange("(j p) f -> p j f", p=128),
                    in_=sh[:, o : o + rows].rearrange("p (j f) -> p j f", f=128),
                )
                nc.gpsimd.collective_compute(
                    "AllGather", mybir.AluOpType.bypass, replica_groups=RG,
                    ins=[bounce.opt()], outs=[agf.opt()],
                )
                hp = hpp.tile([128, rows * NC_], BF16, tag=f"hp{hf}")
                # per-rank pieces so downstream matmuls start on the first
                # ranks while later pieces are still landing
                for c in range(NC_):
                    nc.sync.dma_start(
                        out=hp[:, c * rows : (c + 1) * rows].rearrange(
                            "p (q f) -> p q f", f=128),
                        in_=agf[c * rows : (c + 1) * rows].rearrange(
                            "(q p) f -> p q f", p=128),
                    )
                return hp

            # A-matmul source-chunk (k = global chunk, q = index in half)
            # orders for the two halves.
            HALF_KQ = [
                [(NCH * c + j, len(HALF_CHUNKS[hf]) * c + ji)
                 for c in range(NC_)
                 for ji, j in enumerate(HALF_CHUNKS[hf])]
                for hf in range(2)
            ]

            def a_phase_a(hps, ps):
                """Half-a chunks, all 3 slices, chunk-major: ~21us of PE work
                available as soon as AllGather half-a lands."""
                for i, (k, q) in enumerate(HALF_KQ[0]):
                    for o, n in SLICES:
                        nc.tensor.matmul(
                            out=ps[:, o : o + n],
                            lhsT=hps[0][:, q * 128 : (q + 1) * 128],
                            rhs=asb[:, k * NSH + o : k * NSH + o + n],
                            start=(i == 0), stop=False,
                        )

            def a_phase_b_slice(hps, ps, s):
                """Finish slice s with the half-b chunks."""
                o, n = SLICES[s]
                last = len(HALF_KQ[1]) - 1
                for i, (k, q) in enumerate(HALF_KQ[1]):
                    nc.tensor.matmul(
                        out=ps[:, o : o + n],
                        lhsT=hps[1][:, q * 128 : (q + 1) * 128],
                        rhs=asb[:, k * NSH + o : k * NSH + o + n],
                        start=False, stop=(i == last),
                    )

            def epilogue_slice(ps, hT, s):
                """h[:, slice] = relu(Q) * dinv for one column slice."""
                o, n = SLICES[s]
                rel = wk.tile([128, NSH], F32, tag="rel")
                nc.scalar.activation(rel[:, o : o + n], ps[:, o : o + n], AF.Relu)
                nc.vector.tensor_mul(
                    hT[:, o : o + n], rel[:, o : o + n], dinvbc[:, o : o + n]
                )

            # ---- layer 1 W phase + first AllGathers ----
            sh1 = wk.tile([128, NPAD], BF16, tag="sh1")
            nc.vector.memset(sh1[:, 9 * 128 : NPAD], 0.0)
            for j in HALF_CHUNKS[0]:
                w_chunk(xT, W1s, b1s, sh1, j)
            hp_a = ag_half(sh1, 0, 1)
            for j in HALF_CHUNKS[1]:
                w_chunk(xT, W1s, b1s, sh1, j)
            hp_b = ag_half(sh1, 1, 1)
            hps1 = [hp_a, hp_b]

            # Resident adjacency load, traced here on the Scalar DGE ring:
            # late enough that no early waiter's DMA-sem threshold includes
            # this 12.8MB transfer, early enough (right after the W1
            # epilogues in the ACT stream) to land before the propagate.
            nc.scalar.dma_start(out=asb[:], in_=A_d)

            # ---- layer 1 propagate, pipelined with layer 2 W phase ----
            ps1 = pfm.tile([128, NSH], F32, tag="fm")
            h1T = wk.tile([128, NSH], F32, tag="h1T")
            sh2 = wk.tile([128, NPAD], BF16, tag="sh2")
            nc.vector.memset(sh2[:, 9 * 128 : NPAD], 0.0)
            hps2 = [None, None]
            a_phase_a(hps1, ps1)
            for s in range(3):
                a_phase_b_slice(hps1, ps1, s)
                epilogue_slice(ps1, h1T, s)
                for j in range(NCH):
                    if CHUNK_SLICE[j] == s:
                        w_chunk(h1T, W2s, b2s, sh2, j)
                if s == 0:
                    hps2[0] = ag_half(sh2, 0, 2)
            hps2[1] = ag_half(sh2, 1, 2)

            # ---- layer 2 propagate + full per-slice finalization ----
            # Per column slice: propagate, epilogue, squared column sums,
            # rsqrt normalization, classifier — so nearly the whole tail
            # overlaps the remaining propagate slices.
            ps2 = pfm.tile([128, NSH], F32, tag="fm")
            h2T = wk.tile([128, NSH], F32, tag="h2T")
            sq = wk.tile([128, NSH], F32, tag="sq")
            pns = pns_p.tile([1, NSH], F32, tag="ns")
            sr = wk.tile([1, NSH], F32, tag="sr")
            rr = wk.tile([1, NSH], F32, tag="rr")
            psb = pfm.tile([128, NSH], F32, tag="fm")
            hn = wk.tile([128, NSH], F32, tag="hn")
            oc = wk.tile([128, NCH * C], F32, tag="oc")
            nc.vector.memset(oc[:, 9 * C : NCH * C], 0.0)
            a_phase_a(hps2, ps2)
            for s in range(3):
                o, n = SLICES[s]
                a_phase_b_slice(hps2, ps2, s)
                epilogue_slice(ps2, h2T, s)
                nc.scalar.square(sq[:, o : o + n], h2T[:, o : o + n])
                nc.tensor.matmul(
                    out=pns[:, o : o + n], lhsT=ones_c[:, :],
                    rhs=sq[:, o : o + n], start=True, stop=True,
                )
                nc.scalar.sqrt(sr[:, o : o + n], pns[:, o : o + n])
                nc.vector.tensor_scalar_max(
                    sr[:, o : o + n], sr[:, o : o + n], 1e-12
                )
                nc.vector.reciprocal(rr[:, o : o + n], sr[:, o : o + n])
                nc.tensor.matmul(
                    out=psb[:, o : o + n], lhsT=ones_r[:, :],
                    rhs=rr[:, o : o + n], start=True, stop=True,
                )
                nc.vector.tensor_mul(
                    hn[:, o : o + n], h2T[:, o : o + n], psb[:, o : o + n]
                )
                for j in range(NCH):
                    if CHUNK_SLICE[j] != s:
                        continue
                    mj = 128 if j < 9 else MJ_LAST
                    pc = pnm.tile([128, H], F32, tag="nm")
                    nc.tensor.matmul(
                        out=pc[:mj, :C], lhsT=hn[:, j * 128 : j * 128 + mj],
                        rhs=Wcs, start=True, stop=False,
                    )
                    nc.tensor.matmul(
                        out=pc[:mj, :C], lhsT=ones_r[:, :mj], rhs=bcs,
                        start=False, stop=True,
                    )
                    nc.scalar.copy(oc[:mj, j * C : (j + 1) * C], pc[:mj, :C])
            nc.sync.dma_start(
                out=out_d.rearrange("(j p) c -> p j c", p=128),
                in_=oc[:].rearrange("p (j c) -> p j c", c=C),
            )

    nc.compile()
    return nc


def _prep(inputs):
    x = np.asarray(inputs["x"], np.float32)
    ei = np.asarray(inputs["edge_index"])
    src = ei[0].astype(np.int64)
    dst = ei[1].astype(np.int64)
    loops = np.arange(N, dtype=np.int64)
    s_all = np.concatenate([src, loops])
    d_all = np.concatenate([dst, loops])
    deg = np.bincount(d_all, minlength=N).astype(np.float32)
    gsrc = (s_all // NSH) * NPAD + (s_all % NSH)  # padded global row of source
    dcore = d_all // NSH
    dloc = d_all % NSH

    W1 = np.ascontiguousarray(np.asarray(inputs["W1"], np.float32))
    W2 = np.ascontiguousarray(np.asarray(inputs["W2"], np.float32))
    Wc = np.ascontiguousarray(np.asarray(inputs["Wc"], np.float32))
    b1 = np.asarray(inputs["b1"], np.float32).reshape(1, H)
    b2 = np.asarray(inputs["b2"], np.float32).reshape(1, H)
    bc = np.asarray(inputs["bc"], np.float32).reshape(1, C)

    in_maps = []
    for c in range(NC_):
        m = dcore == c
        flat = gsrc[m] * NSH + dloc[m]
        Ac = np.bincount(flat, minlength=NFULL * NSH).astype(np.float32)
        Ac = Ac.reshape(KCH, 128, NSH).transpose(1, 0, 2)
        Ac = np.ascontiguousarray(Ac).reshape(128, KCH * NSH)
        degc = deg[c * NSH : (c + 1) * NSH]
        degpp = np.concatenate([degc, np.ones(NPAD - NSH, np.float32)])
        degpp = degpp.reshape(NCH, 128).T
        pk = np.zeros((128, PK_COLS), np.float32)
        pk[:, PK_W1 : PK_W1 + H] = W1
        pk[:, PK_W2 : PK_W2 + H] = W2
        pk[:, PK_WC : PK_WC + C] = Wc
        pk[:, PK_DEGPP : PK_DEGPP + NCH] = degpp
        pk[0, PK_B1 : PK_B1 + H] = b1[0]
        pk[0, PK_B2 : PK_B2 + H] = b2[0]
        pk[0, PK_BC : PK_BC + C] = bc[0]
        pk[0, PK_DEGROW : PK_DEGROW + NSH] = degc
        in_maps.append({
            "xT": np.ascontiguousarray(x[c * NSH : (c + 1) * NSH].T),
            "A": Ac.astype(ml_dtypes.float8_e4m3),
            "packed": pk,
        })
    return in_maps


def run(inputs, **spmd_kwargs):
    from concourse import bass_utils

    if "nc" not in _cache:
        _cache["nc"] = _build()
    in_maps = _prep(inputs)
    res = bass_utils.run_bass_kernel_spmd(
        _cache["nc"], in_maps, core_ids=list(range(NC_)), **spmd_kwargs
    )
    out = np.concatenate(
        [np.asarray(res.results[c]["out"])[:NSH] for c in range(NC_)], axis=0
    )
    return out.astype(np.float32), res


def kernel(**inputs):
    out, _ = run(inputs)
    return out
